# revision 50
# baseline (speedup 1.0000x reference)
"""MoE layer (B=4, N=2048, C=1024, F=4096, E=8, top-2) on 8 trn2 NeuronCores.

Sharding (fast path, b1 == b2 == 0 — the benchmarked case): F-parallel
over all experts.  The host computes the (tiny) router and builds ONE
gated, expert-major token stream shared by all cores; core d holds
f-blocks [4d, 4d+4) of EVERY expert's w1/w2 (same 16.8 MB SBUF footprint
as one full expert) and computes the partial FFN
    y_partial = relu(xg @ w1[fslice].T) @ w2[:, fslice].T
for ALL token-expert pairs.  The host sums the 8 partials per token.

Why F-parallel: every core runs the exact same instruction stream over
the exact same segment sizes (the true per-expert token counts padded
to 128), so per-core work is sum_e pad(n_e)/8 instead of max_e pad(n_e)
— the expert load imbalance vanishes instead of padding every core to
the hottest expert's count.  All matmuls in bf16 (1 cycle/row on the
PE); tokens pre-gated on host; y accumulates in fp32 PSUM across the 4
resident f-blocks of each chunk and retires once, in bf16.
"""

import numpy as np

P = 128
C = 1024
F = 4096
E = 8
NFB = 4  # f-blocks per core (32 total / 8 cores)
SCH = 384  # token chunk: 3 PSUM banks x 2 C-halves for y + 2 for h = 8
NWARM = 12  # PE warm-up matmuls: a contiguous >=3.4us busy window inside
# the block un-throttles the HAM clock gate (1.2 -> 2.4 GHz) BEFORE real
# matmuls start, and the block ends (~12.9us) right as the first token
# and weight DMAs become consumable — any PE gap between warm-up and the
# real stream would restart the HAM's free-running busy window and leave
# the first real matmuls at half clock


def _build(cap: int):
    """Slow fallback (nonzero biases): expert-parallel, fp32 weights."""
    import concourse.mybir as mybir
    from concourse import bacc
    from concourse.tile import TileContext

    f32 = mybir.dt.float32
    f32r = mybir.dt.float32r
    nS = cap // SCH
    nc = bacc.Bacc(None, target_bir_lowering=False)

    xgT = nc.dram_tensor("xgT", [C, cap], f32, kind="ExternalInput")
    w1t = nc.dram_tensor("w1t", [C, F], f32, kind="ExternalInput")
    w2t = nc.dram_tensor("w2t", [F, C], f32, kind="ExternalInput")
    b1r = nc.dram_tensor("b1r", [P, F // P], f32, kind="ExternalInput")
    b2r = nc.dram_tensor("b2r", [P, C], f32, kind="ExternalInput")
    wg = nc.dram_tensor("wg", [P, cap // P], f32, kind="ExternalInput")
    yg = nc.dram_tensor("yg", [cap, C], f32, kind="ExternalOutput")

    w1v = w1t.ap().rearrange("(co ci) f -> ci co f", ci=P)  # [128, 8, F]
    xgv = xgT.ap().rearrange("(co ci) n -> ci co n", ci=P)  # [128, 8, cap]

    with TileContext(nc) as tc:
        with (
            tc.tile_pool(name="consts", bufs=1) as consts,
            tc.tile_pool(name="wpool", bufs=4) as wpool,
            tc.tile_pool(name="xpool", bufs=2) as xpool,
            tc.tile_pool(name="hpool", bufs=3) as hpool,
            tc.tile_pool(name="ypool", bufs=3) as ypool,
            tc.tile_pool(name="psum_h", bufs=2, space="PSUM") as psum_h,
            tc.tile_pool(name="psum_y", bufs=1, space="PSUM") as psum_y,
        ):
            b1_sb = consts.tile([P, F // P], f32)
            nc.sync.dma_start(b1_sb[:], b1r[:, :])
            b2_sb = consts.tile([P, C], f32)
            nc.sync.dma_start(b2_sb[:], b2r[:, :])
            wg_sb = consts.tile([P, cap // P], f32)
            nc.sync.dma_start(wg_sb[:], wg[:, :])

            for s in range(nS):
                xg_s = xpool.tile([P, 8, SCH], f32r, tag="xg")
                nc.sync.dma_start(xg_s[:], xgv[:, :, s * SCH : (s + 1) * SCH].bitcast(f32r))

                yps = [
                    [
                        psum_y.tile(
                            [P, 512], f32, tag=f"y_{t}_{cc}", name=f"y_{t}_{cc}"
                        )
                        for cc in range(2)
                    ]
                    for t in range(3)
                ]

                for f in range(F // P):  # 32
                    w1c = wpool.tile([P, 8, P], f32r, tag="w1c")
                    nc.sync.dma_start(w1c[:], w1v[:, :, f * P : (f + 1) * P].bitcast(f32r))
                    w2c = wpool.tile([P, C], f32r, tag="w2c")
                    nc.sync.dma_start(w2c[:], w2t[f * P : (f + 1) * P, :].bitcast(f32r))

                    hps = psum_h.tile([P, SCH], f32, tag="h")
                    for c in range(8):
                        nc.tensor.matmul(
                            hps[:],
                            lhsT=w1c[:, c, :],
                            rhs=xg_s[:, c, :],
                            start=(c == 0),
                            stop=(c == 7),
                        )
                    hT = hpool.tile([P, SCH], f32r, tag="hT")
                    nc.scalar.activation(
                        hT[:],
                        hps[:],
                        mybir.ActivationFunctionType.Relu,
                        bias=b1_sb[:, f : f + 1],
                        scale=1.0,
                    )
                    for t in range(3):
                        for cc in range(2):
                            nc.tensor.matmul(
                                yps[t][cc][:],
                                lhsT=hT[:, t * P : (t + 1) * P],
                                rhs=w2c[:, cc * 512 : (cc + 1) * 512],
                                start=(f == 0),
                                stop=(f == F // P - 1),
                            )

                for t in range(3):
                    y_sb = ypool.tile([P, C], f32, tag="y_sb")
                    for cc in range(2):
                        sl = slice(cc * 512, (cc + 1) * 512)
                        nc.vector.tensor_add(y_sb[:, sl], yps[t][cc][:], b2_sb[:, sl])
                    yf = ypool.tile([P, C], f32, tag="yf")
                    nc.scalar.mul(yf[:], y_sb[:], wg_sb[:, s * 3 + t : s * 3 + t + 1])
                    nc.sync.dma_start(
                        yg[(s * 3 + t) * P : (s * 3 + t + 1) * P, :], yf[:]
                    )
    nc.compile()
    return nc


def _chunks(m):
    """Split a (128-multiple) segment into chunks of SCH, runt last.

    Chunks below 256 tokens are mm1 LDWEIGHTS-bound on the PE (weight
    load 107 ns/c-block vs ~51 ns of 128-col matmul), so a trailing
    128-token runt is rebalanced to [256, 256] — same mm2 tile count.
    """
    sizes = [SCH] * (m // SCH)
    rem = m - SCH * len(sizes)
    if rem:
        sizes.append(rem)
    if len(sizes) >= 2 and sizes[-1] == P:
        sizes[-2:] = [SCH - P, SCH - P + sizes[-1] - P]
    return sizes


def _build_fast(ms: tuple, ns: tuple):
    """Fast path (b1 == 0 and b2 == 0): F-parallel over all experts.

    ms[e] = padded token count of expert e (multiple of 128; same on all
    cores); ns[e] = exact token count.  Per core inputs:
      xgf [sum(ms)*C]        bf16 gated tokens, expert-major, per-chunk
                             [ci, co, n] tiles (identical on all cores)
      w1p [E, 4, 128, 8, 128] bf16 w1[e][fslice].T tiled for mm1 lhsT
      w2p [E, 4, 128, 1024]   bf16 w2[e][:, fslice].T tiled for mm2 rhs
    output:
      yg  [sum(ms), 1024] bf16 partial y (this core's f-slice term)

    All weights stay resident in SBUF (128 KB/partition); each token
    chunk's y accumulates in PSUM across the 4 f-blocks and retires once.
    """
    import concourse.mybir as mybir
    from concourse import bacc
    from concourse.tile import TileContext

    f32 = mybir.dt.float32
    bf16 = mybir.dt.bfloat16
    Mtot = sum(ms)  # padded rows (yg layout)
    Mx = sum(ns)  # exact rows (xgf layout, chunk blocks packed tight)
    nc = bacc.Bacc(None, target_bir_lowering=False)

    xgf = nc.dram_tensor("xgf", [Mx * C], bf16, kind="ExternalInput")
    w1p = nc.dram_tensor("w1p", [E, NFB, P, 8, P], bf16, kind="ExternalInput")
    w2p = nc.dram_tensor("w2p", [E, NFB, P, C], bf16, kind="ExternalInput")
    yg = nc.dram_tensor("yg", [Mtot, C], bf16, kind="ExternalOutput")

    with TileContext(nc) as tc:
        with (
            tc.tile_pool(name="warm", bufs=1) as warm,
            tc.tile_pool(name="wpool", bufs=1) as wpool,
            tc.tile_pool(name="xpool", bufs=4) as xpool,
            tc.tile_pool(name="hpool", bufs=4) as hpool,
            tc.tile_pool(name="ypool", bufs=3) as ypool,
            tc.tile_pool(name="psum_h", bufs=2, space="PSUM") as psum_h,
            tc.tile_pool(name="psum_y", bufs=1, space="PSUM") as psum_y,
        ):
            # PE warm-up: dummy matmuls on memset tiles keep the PE busy
            # from t~0 so the HAM clock gate un-throttles (1.2 -> 2.4 GHz)
            # while the first weight/token DMAs are still in flight (DMA
            # cold-start means nothing lands before ~8 us regardless of
            # size, so real matmuls cannot start earlier anyway).
            warm_w = warm.tile([P, P], bf16, name="warm_w")
            warm_x = warm.tile([P, SCH], bf16, name="warm_x")
            nc.vector.memset(warm_w[:], 0.0)
            nc.vector.memset(warm_x[:], 0.0)
            warm_ps = psum_h.tile([P, SCH], f32, tag="h", name="warm_ps")
            for _ in range(NWARM):
                nc.tensor.matmul(
                    warm_ps[:], lhsT=warm_w[:], rhs=warm_x[:], start=True, stop=True
                )

            # (expert, chunk) schedule, expert-major.  Chunk sizes carry
            # the exact token count (nx <= padded sz): mm1 streams only
            # real tokens; mm2/retire tiles are partition-sliced to them.
            sched = []  # (e, y_row_offset, xgf_row_offset, padded, exact)
            off = 0
            xoff = 0
            for e in range(E):
                coff = 0
                left = ns[e]  # exact tokens still unprocessed
                for sz in _chunks(ms[e]):
                    nx = min(left, sz)
                    sched.append((e, off + coff, xoff, sz, nx))
                    coff += sz
                    xoff += nx
                    left -= nx
                off += ms[e]

            def load_xg(si):
                e, row0, xoff, sz, nx = sched[si]
                xg_s = xpool.tile([P, 8, nx], bf16, tag="xg", name="xg_s")
                src = xgf[xoff * C : (xoff + nx) * C]
                v = src.rearrange("(ci co n) -> ci co n", ci=P, co=8)
                nc.sync.dma_start(xg_s[:], v)
                return xg_s

            w1g = wpool.tile([P, E, NFB, 8, P], bf16, tag="w1g", name="w1g")
            w2g = wpool.tile([P, E, NFB, C], bf16, tag="w2g", name="w2g")

            loaded1 = [False] * E  # w1 slices issued (whole expert)
            loaded2 = [False] * E  # w2 slices issued (whole expert)

            # one bulk dma_start per expert per weight tensor: each
            # dma_start occupies the in-order issue queue for ~0.6 us, so
            # fewer, bigger loads get the critical early bytes moving
            # sooner (expert 0's w1 is split once so the very first
            # matmul only waits on its first f-block)
            def load_w1(e):
                if not loaded1[e]:
                    loaded1[e] = True
                    nc.sync.dma_start(w1g[:, e], w1p[e])

            def load_w2(e):
                if not loaded2[e]:
                    loaded2[e] = True
                    nc.sync.dma_start(w2g[:, e], w2p[e])

            # DMA issue order matters: all loads drain through ONE in-order
            # hardware queue, so interleave token-chunk prefetches with
            # weight loads in consumption order.  Issuing all 16.8 MB of
            # weights up front starves the per-chunk token DMAs behind
            # them and stalls the PE for ~40 us.  Prologue: chunk 0's
            # 262 KB first, then expert 0's f-blocks interleaved with the
            # next chunks; afterwards expert e+1's four pairs trickle out
            # one per chunk during expert e's segment.
            # w1 slices first: mm2 (and so w2) runs two f-blocks behind
            # mm1, so w2's bytes aren't needed until ~3 us later — keep
            # them out of the scarce cold-DMA window.
            xg_q = [load_xg(0)]
            loaded1[0] = True
            nc.sync.dma_start(w1g[:, 0, 0], w1p[0, 0])
            nc.sync.dma_start(w1g[:, 0, 1:NFB], w1p[0, 1:NFB])
            if len(sched) > 1:
                xg_q.append(load_xg(1))
            load_w2(0)
            if len(sched) > 2:
                xg_q.append(load_xg(2))
            PREF = 3  # xg prefetch depth (xpool bufs = PREF + 1)

            for si, (e, row0, xoff, sz, nx) in enumerate(sched):
                if si == 0 or sched[si - 1][0] != e:
                    load_w1(e)  # safety: must be resident now
                    load_w2(e)
                    seg_chunk = 0
                else:
                    seg_chunk += 1
                nt = sz // P  # mm2/retire always use full 128-token tiles:
                # partial-height DMAs degrade to one engine — pad hT instead
                xg_s = xg_q.pop(0)
                if si + PREF < len(sched):
                    xg_q.append(load_xg(si + PREF))
                if e + 1 < E:
                    if seg_chunk == 0:
                        load_w1(e + 1)
                    elif seg_chunk == 1:
                        load_w2(e + 1)

                yps = [
                    psum_y.tile([P, C], f32, tag=f"y_{t}", name=f"y_{t}")
                    for t in range(nt)
                ]

                final_chunk = si == len(sched) - 1

                def retire_tile(t, row0=row0, yps=yps, split=False):
                    r = row0 + t * P
                    yf = ypool.tile([P, C], bf16, tag=f"yf{t % 2}", name="yf")
                    if split:
                        # tail shave for the very last tile: copy each
                        # 512-col half on a separate engine (DVE || ACT) as
                        # soon as ITS accumulation closes, one full DMA
                        nc.vector.tensor_copy(yf[:, 0:512], yps[t][:, 0:512])
                        nc.scalar.activation(
                            yf[:, 512:C], yps[t][:, 512:C],
                            mybir.ActivationFunctionType.Copy,
                        )
                    elif t % 2 == 0:
                        # single copy per tile, alternating DVE / ACT so
                        # neighboring tiles drain in parallel
                        nc.vector.tensor_copy(yf[:], yps[t][:])
                    else:
                        nc.scalar.activation(
                            yf[:], yps[t][:], mybir.ActivationFunctionType.Copy
                        )
                    nc.sync.dma_start(yg[r : r + P, :], yf[:])

                def mm2(fl, hT, last=False, yps=yps, nt=nt, e=e,
                        final_chunk=final_chunk):
                    for t in range(nt):
                        split = last and final_chunk and t == nt - 1
                        for cc in range(2):
                            nc.tensor.matmul(
                                yps[t][:, cc * 512 : (cc + 1) * 512],
                                lhsT=hT[:, t * P : (t + 1) * P],
                                rhs=w2g[:, e, fl, cc * 512 : (cc + 1) * 512],
                                start=(fl == 0),
                                stop=(fl == NFB - 1),
                            )
                        if last:
                            # retire as soon as this tile's accumulation
                            # closes: frees its PSUM banks for the next
                            # chunk's mm2 that much earlier
                            retire_tile(t, split=split)

                # software pipeline: mm2 runs two fl behind mm1, so the
                # relu feeding each mm2 block retired long before the PE
                # reaches it; the last two mm2 blocks + the PSUM retires
                # drain after the next chunk's first mm1 blocks.  Chunk 0
                # lags a full NFB so its mm2 (and so its w2 slices) start
                # only after all of mm1 — w2 bytes stay out of the scarce
                # cold-DMA window.
                lag = NFB if si == 0 else 2
                hTs = []
                for fl in range(NFB):
                    hps = psum_h.tile([P, SCH], f32, tag="h", name="hps")
                    for c in range(8):
                        nc.tensor.matmul(
                            hps[:, :nx],
                            lhsT=w1g[:, e, fl, c, :],
                            rhs=xg_s[:, c, :],
                            start=(c == 0),
                            stop=(c == 7),
                        )
                    hT = hpool.tile([P, SCH], bf16, tag="hT", name="hT")
                    if nx < sz:
                        # runt chunk: mm1 streamed only the nx real tokens;
                        # zero hT's pad columns so mm2 runs full 128-tiles
                        # (partial-height DMAs/psum would be slower)
                        nc.vector.memset(hT[:, nx:sz], 0.0)
                    if fl >= NFB - 2:
                        # last fl's: per-token-tile relu so mm2(t) can
                        # start as soon as its slice is ready
                        for t in range(nt):
                            tl = slice(t * P, min((t + 1) * P, nx))
                            if tl.start >= nx:
                                break
                            nc.scalar.activation(
                                hT[:, tl],
                                hps[:, tl],
                                mybir.ActivationFunctionType.Relu,
                            )
                    else:
                        nc.scalar.activation(
                            hT[:, :nx],
                            hps[:, :nx],
                            mybir.ActivationFunctionType.Relu,
                        )
                    hTs.append(hT)
                    if fl >= lag:
                        mm2(fl - lag, hTs[fl - lag])
                for k in range(NFB - min(lag, NFB), NFB):
                    mm2(k, hTs[k], last=(k == NFB - 1))
    nc.compile()
    return nc


_CACHE = {}
_TRACE = False  # test harness sets True to capture an NTFF profile
_LAST_RES = None


def _get_nc(key, builder):
    if key not in _CACHE:
        _CACHE[key] = builder()
    return _CACHE[key]


def _route(x_flat, router_w):
    """Top-2 routing, float64 for stable selection. Returns idx/weights per expert."""
    logits = x_flat.astype(np.float64) @ router_w.astype(np.float64).T
    t = np.exp(logits - logits.max(-1, keepdims=True))
    p = t / t.sum(-1, keepdims=True)
    top2 = np.argsort(-p, axis=-1)[:, :2]
    pv = np.take_along_axis(p, top2, axis=-1)
    wn = pv / (pv.sum(-1, keepdims=True) + 1e-9)
    return top2, wn


def kernel(x, router_w, w1, b1, w2, b2):
    import ml_dtypes
    from concourse.bass_utils import run_bass_kernel_spmd

    bf16 = ml_dtypes.bfloat16
    Bx, Nx, Cx = x.shape
    x_flat = np.ascontiguousarray(x.reshape(-1, Cx))
    T = x_flat.shape[0]

    top2, wn = _route(x_flat, router_w)
    idxs, gates = [], []
    for e in range(E):
        sel = top2 == e
        we = np.where(sel, wn, 0.0).sum(-1)
        idx = np.nonzero(sel.any(-1))[0]
        idxs.append(idx)
        gates.append(we[idx].astype(np.float32))

    fast = bool(np.all(b1 == 0) and np.all(b2 == 0))
    global _LAST_RES

    if not fast:
        cap = max(len(i) for i in idxs)
        cap = ((cap + SCH - 1) // SCH) * SCH
        nc = _get_nc(("slow", cap), lambda: _build(cap))
        in_maps = []
        for e in range(E):
            n_e = len(idxs[e])
            xg = np.zeros((cap, Cx), np.float32)
            xg[:n_e] = x_flat[idxs[e]]
            wg = np.zeros(cap, np.float32)
            wg[:n_e] = gates[e]
            in_maps.append(
                {
                    "xgT": np.ascontiguousarray(xg.T),
                    "w1t": np.ascontiguousarray(w1[e].T),
                    "w2t": np.ascontiguousarray(w2[e].T),
                    "b1r": np.ascontiguousarray(b1[e].reshape(F // P, P).T),
                    "b2r": np.ascontiguousarray(np.broadcast_to(b2[e], (P, Cx))),
                    "wg": np.ascontiguousarray(wg.reshape(cap // P, P).T),
                }
            )
        res = run_bass_kernel_spmd(nc, in_maps, core_ids=list(range(E)), trace=_TRACE)
        _LAST_RES = res
        out = np.zeros((T, Cx), np.float32)
        for e in range(E):
            n_e = len(idxs[e])
            out[idxs[e]] += res.results[e]["yg"][:n_e].astype(np.float32)
        return out.reshape(Bx, Nx, Cx)

    # ---- fast path: F-parallel over all experts ----
    # order experts so the very last chunk is as small as possible (the
    # final retire + output DMA is the kernel's tail)
    ms_nat = [((len(idxs[e]) + P - 1) // P) * P for e in range(E)]
    order = sorted(range(E), key=lambda e: (-(ms_nat[e] % SCH or SCH), e))
    ms = tuple(ms_nat[e] for e in order)
    ns = tuple(len(idxs[e]) for e in order)
    nc = _get_nc(("fast", ms, ns), lambda: _build_fast(ms, ns))

    # shared gated token stream, expert-major, per-chunk [ci, co, n] tiles,
    # chunk blocks packed tight (exact sizes, no pad rows)
    blocks = []
    for i, e in enumerate(order):
        n_e = len(idxs[e])
        xgb = (x_flat[idxs[e]] * gates[e][:, None]).astype(bf16)  # pre-gate
        off = 0
        left = n_e
        for sz in _chunks(ms_nat[e]):
            nx = min(left, sz)
            blocks.append(
                np.ascontiguousarray(
                    xgb[off : off + nx].reshape(nx, 8, P).transpose(2, 1, 0)
                ).ravel()
            )
            off += nx
            left -= nx
    xgf = np.concatenate(blocks)

    # per-core weight slices: core d holds f-blocks [4d, 4d+4) of every expert
    # w1 tiled:  w1t[e][fb, fo, c, ci] -> lhsT layout [ci, c, fo]
    w1t = [
        w1[e].reshape(F // P, P, 8, P).transpose(0, 3, 2, 1).astype(bf16)
        for e in order
    ]
    w2t = [w2[e].T.reshape(F // P, P, Cx).astype(bf16) for e in order]
    in_maps = []
    for d in range(8):
        fsl = slice(NFB * d, NFB * (d + 1))
        w1pd = np.ascontiguousarray(np.stack([w1t[i][fsl] for i in range(E)]))
        w2pd = np.ascontiguousarray(np.stack([w2t[i][fsl] for i in range(E)]))
        in_maps.append({"xgf": xgf, "w1p": w1pd, "w2p": w2pd})

    res = run_bass_kernel_spmd(nc, in_maps, core_ids=list(range(8)), trace=_TRACE)
    _LAST_RES = res

    # host combine: sum the 8 partial-y streams, then scatter-add per expert
    ysum = res.results[0]["yg"].astype(np.float32)
    for d in range(1, 8):
        ysum += res.results[d]["yg"].astype(np.float32)
    out = np.zeros((T, Cx), np.float32)
    off = 0
    for i, e in enumerate(order):
        n_e = len(idxs[e])
        out[idxs[e]] += ysum[off : off + n_e]
        off += ms[i]
    return out.reshape(Bx, Nx, Cx)


# revision 53
# speedup vs baseline: 1.1979x; 1.1979x over previous
"""MoE layer (B=4, N=2048, C=1024, F=4096, E=8, top-2) on 8 trn2 NeuronCores.

Sharding (fast path, b1 == b2 == 0 — the benchmarked case): F-parallel
over all experts.  The host computes the (tiny) router and builds ONE
gated, expert-major token stream shared by all cores; core d holds
f-blocks [4d, 4d+4) of EVERY expert's w1/w2 (same 16.8 MB SBUF footprint
as one full expert) and computes the partial FFN
    y_partial = relu(xg @ w1[fslice].T) @ w2[:, fslice].T
for ALL token-expert pairs.  The host sums the 8 partials per token.

Why F-parallel: every core runs the exact same instruction stream over
the exact same segment sizes (the true per-expert token counts padded
to 128), so per-core work is sum_e pad(n_e)/8 instead of max_e pad(n_e)
— the expert load imbalance vanishes instead of padding every core to
the hottest expert's count.  All matmuls in bf16 (1 cycle/row on the
PE); tokens pre-gated on host; y accumulates in fp32 PSUM across the 4
resident f-blocks of each chunk and retires once, in bf16.
"""

import numpy as np

P = 128
C = 1024
F = 4096
E = 8
NFB = 4  # f-blocks per core (32 total / 8 cores)
SCH = 384  # token chunk: 3 PSUM banks x 2 C-halves for y + 2 for h = 8
NWARM = 12  # PE warm-up matmuls: a contiguous >=3.4us busy window inside
# the block un-throttles the HAM clock gate (1.2 -> 2.4 GHz) BEFORE real
# matmuls start, and the block ends (~12.9us) right as the first token
# and weight DMAs become consumable — any PE gap between warm-up and the
# real stream would restart the HAM's free-running busy window and leave
# the first real matmuls at half clock


def _build(cap: int):
    """Slow fallback (nonzero biases): expert-parallel, fp32 weights."""
    import concourse.mybir as mybir
    from concourse import bacc
    from concourse.tile import TileContext

    f32 = mybir.dt.float32
    f32r = mybir.dt.float32r
    nS = cap // SCH
    nc = bacc.Bacc(None, target_bir_lowering=False)

    xgT = nc.dram_tensor("xgT", [C, cap], f32, kind="ExternalInput")
    w1t = nc.dram_tensor("w1t", [C, F], f32, kind="ExternalInput")
    w2t = nc.dram_tensor("w2t", [F, C], f32, kind="ExternalInput")
    b1r = nc.dram_tensor("b1r", [P, F // P], f32, kind="ExternalInput")
    b2r = nc.dram_tensor("b2r", [P, C], f32, kind="ExternalInput")
    wg = nc.dram_tensor("wg", [P, cap // P], f32, kind="ExternalInput")
    yg = nc.dram_tensor("yg", [cap, C], f32, kind="ExternalOutput")

    w1v = w1t.ap().rearrange("(co ci) f -> ci co f", ci=P)  # [128, 8, F]
    xgv = xgT.ap().rearrange("(co ci) n -> ci co n", ci=P)  # [128, 8, cap]

    with TileContext(nc) as tc:
        with (
            tc.tile_pool(name="consts", bufs=1) as consts,
            tc.tile_pool(name="wpool", bufs=4) as wpool,
            tc.tile_pool(name="xpool", bufs=2) as xpool,
            tc.tile_pool(name="hpool", bufs=3) as hpool,
            tc.tile_pool(name="ypool", bufs=3) as ypool,
            tc.tile_pool(name="psum_h", bufs=2, space="PSUM") as psum_h,
            tc.tile_pool(name="psum_y", bufs=1, space="PSUM") as psum_y,
        ):
            b1_sb = consts.tile([P, F // P], f32)
            nc.sync.dma_start(b1_sb[:], b1r[:, :])
            b2_sb = consts.tile([P, C], f32)
            nc.sync.dma_start(b2_sb[:], b2r[:, :])
            wg_sb = consts.tile([P, cap // P], f32)
            nc.sync.dma_start(wg_sb[:], wg[:, :])

            for s in range(nS):
                xg_s = xpool.tile([P, 8, SCH], f32r, tag="xg")
                nc.sync.dma_start(xg_s[:], xgv[:, :, s * SCH : (s + 1) * SCH].bitcast(f32r))

                yps = [
                    [
                        psum_y.tile(
                            [P, 512], f32, tag=f"y_{t}_{cc}", name=f"y_{t}_{cc}"
                        )
                        for cc in range(2)
                    ]
                    for t in range(3)
                ]

                for f in range(F // P):  # 32
                    w1c = wpool.tile([P, 8, P], f32r, tag="w1c")
                    nc.sync.dma_start(w1c[:], w1v[:, :, f * P : (f + 1) * P].bitcast(f32r))
                    w2c = wpool.tile([P, C], f32r, tag="w2c")
                    nc.sync.dma_start(w2c[:], w2t[f * P : (f + 1) * P, :].bitcast(f32r))

                    hps = psum_h.tile([P, SCH], f32, tag="h")
                    for c in range(8):
                        nc.tensor.matmul(
                            hps[:],
                            lhsT=w1c[:, c, :],
                            rhs=xg_s[:, c, :],
                            start=(c == 0),
                            stop=(c == 7),
                        )
                    hT = hpool.tile([P, SCH], f32r, tag="hT")
                    nc.scalar.activation(
                        hT[:],
                        hps[:],
                        mybir.ActivationFunctionType.Relu,
                        bias=b1_sb[:, f : f + 1],
                        scale=1.0,
                    )
                    for t in range(3):
                        for cc in range(2):
                            nc.tensor.matmul(
                                yps[t][cc][:],
                                lhsT=hT[:, t * P : (t + 1) * P],
                                rhs=w2c[:, cc * 512 : (cc + 1) * 512],
                                start=(f == 0),
                                stop=(f == F // P - 1),
                            )

                for t in range(3):
                    y_sb = ypool.tile([P, C], f32, tag="y_sb")
                    for cc in range(2):
                        sl = slice(cc * 512, (cc + 1) * 512)
                        nc.vector.tensor_add(y_sb[:, sl], yps[t][cc][:], b2_sb[:, sl])
                    yf = ypool.tile([P, C], f32, tag="yf")
                    nc.scalar.mul(yf[:], y_sb[:], wg_sb[:, s * 3 + t : s * 3 + t + 1])
                    nc.sync.dma_start(
                        yg[(s * 3 + t) * P : (s * 3 + t + 1) * P, :], yf[:]
                    )
    nc.compile()
    return nc


def _chunks(m):
    """Split a (128-multiple) segment into chunks of SCH, runt last.

    Chunks below 256 tokens are mm1 LDWEIGHTS-bound on the PE (weight
    load 107 ns/c-block vs ~51 ns of 128-col matmul), so a trailing
    128-token runt is rebalanced to [256, 256] — same mm2 tile count.
    """
    sizes = [SCH] * (m // SCH)
    rem = m - SCH * len(sizes)
    if rem:
        sizes.append(rem)
    if len(sizes) >= 2 and sizes[-1] == P:
        sizes[-2:] = [SCH - P, SCH - P + sizes[-1] - P]
    return sizes


def _build_fast(ms: tuple, ns: tuple):
    """Fast path (b1 == 0 and b2 == 0): F-parallel over all experts.

    ms[e] = padded token count of expert e (multiple of 128; same on all
    cores); ns[e] = exact token count.  Per core inputs:
      xgf [sum(ms)*C]        bf16 gated tokens, expert-major, per-chunk
                             [ci, co, n] tiles (identical on all cores)
      w1p [E, 4, 128, 8, 128] bf16 w1[e][fslice].T tiled for mm1 lhsT
      w2p [E, 4, 128, 1024]   bf16 w2[e][:, fslice].T tiled for mm2 rhs
    output:
      yg  [sum(ms), 1024] bf16 partial y (this core's f-slice term)

    All weights stay resident in SBUF (128 KB/partition); each token
    chunk's y accumulates in PSUM across the 4 f-blocks and retires once.
    """
    import concourse.mybir as mybir
    from concourse import bacc
    from concourse.tile import TileContext

    f32 = mybir.dt.float32
    bf16 = mybir.dt.bfloat16
    Mtot = sum(ms)  # padded rows (yg layout)
    Mx = sum(ns)  # exact rows (xgf layout, chunk blocks packed tight)
    nc = bacc.Bacc(None, target_bir_lowering=False)

    xgf = nc.dram_tensor("xgf", [Mx * C], bf16, kind="ExternalInput")
    # weight layouts are partition(=ci/fi)-major per expert so a whole
    # expert loads with ONE dma_start
    w1p = nc.dram_tensor("w1p", [E, P, NFB, 8, P], bf16, kind="ExternalInput")
    w2p = nc.dram_tensor("w2p", [E, P, NFB, C], bf16, kind="ExternalInput")
    yg = nc.dram_tensor("yg", [Mtot, C], bf16, kind="ExternalOutput")

    with TileContext(nc) as tc:
        with (
            tc.tile_pool(name="warm", bufs=1) as warm,
            tc.tile_pool(name="wpool", bufs=1) as wpool,
            tc.tile_pool(name="xpool", bufs=4) as xpool,
            tc.tile_pool(name="hpool", bufs=4) as hpool,
            tc.tile_pool(name="ypool", bufs=3) as ypool,
            tc.tile_pool(name="psum_h", bufs=2, space="PSUM") as psum_h,
            tc.tile_pool(name="psum_y", bufs=1, space="PSUM") as psum_y,
        ):
            # PE warm-up: dummy matmuls on memset tiles keep the PE busy
            # from t~0 so the HAM clock gate un-throttles (1.2 -> 2.4 GHz)
            # while the first weight/token DMAs are still in flight (DMA
            # cold-start means nothing lands before ~8 us regardless of
            # size, so real matmuls cannot start earlier anyway).
            warm_w = warm.tile([P, P], bf16, name="warm_w")
            warm_x = warm.tile([P, SCH], bf16, name="warm_x")
            nc.vector.memset(warm_w[:], 0.0)
            nc.vector.memset(warm_x[:], 0.0)
            warm_ps = psum_h.tile([P, SCH], f32, tag="h", name="warm_ps")
            for _ in range(NWARM):
                nc.tensor.matmul(
                    warm_ps[:], lhsT=warm_w[:], rhs=warm_x[:], start=True, stop=True
                )

            # (expert, chunk) schedule, expert-major.  Chunk sizes carry
            # the exact token count (nx <= padded sz): mm1 streams only
            # real tokens; mm2/retire tiles are partition-sliced to them.
            sched = []  # (e, y_row_offset, xgf_row_offset, padded, exact)
            off = 0
            xoff = 0
            for e in range(E):
                coff = 0
                left = ns[e]  # exact tokens still unprocessed
                for sz in _chunks(ms[e]):
                    nx = min(left, sz)
                    sched.append((e, off + coff, xoff, sz, nx))
                    coff += sz
                    xoff += nx
                    left -= nx
                off += ms[e]

            def load_xg(si):
                e, row0, xoff, sz, nx = sched[si]
                xg_s = xpool.tile([P, 8, nx], bf16, tag="xg", name="xg_s")
                src = xgf[xoff * C : (xoff + nx) * C]
                v = src.rearrange("(ci co n) -> ci co n", ci=P, co=8)
                nc.sync.dma_start(xg_s[:], v)
                return xg_s

            w1g = wpool.tile([P, E, NFB, 8, P], bf16, tag="w1g", name="w1g")
            w2g = wpool.tile([P, E, NFB, C], bf16, tag="w2g", name="w2g")

            loaded1 = [False] * E  # w1 slices issued (whole expert)
            loaded2 = [False] * E  # w2 slices issued (whole expert)

            # one bulk dma_start per expert per weight tensor: each
            # dma_start occupies the in-order issue queue for ~0.6 us, so
            # fewer, bigger loads get the critical early bytes moving
            # sooner (expert 0's w1 is split once so the very first
            # matmul only waits on its first f-block)
            def load_w1(e):
                if not loaded1[e]:
                    loaded1[e] = True
                    nc.sync.dma_start(w1g[:, e], w1p[e])

            def load_w2(e):
                if not loaded2[e]:
                    loaded2[e] = True
                    nc.sync.dma_start(w2g[:, e], w2p[e])

            # DMA issue order matters: all loads drain through ONE in-order
            # hardware queue, so interleave token-chunk prefetches with
            # weight loads in consumption order.  Issuing all 16.8 MB of
            # weights up front starves the per-chunk token DMAs behind
            # them and stalls the PE for ~40 us.  Prologue: chunk 0's
            # 262 KB first, then expert 0's f-blocks interleaved with the
            # next chunks; afterwards expert e+1's four pairs trickle out
            # one per chunk during expert e's segment.
            # w1 slices first: mm2 (and so w2) runs two f-blocks behind
            # mm1, so w2's bytes aren't needed until ~3 us later — keep
            # them out of the scarce cold-DMA window.
            xg_q = [load_xg(0)]
            loaded1[0] = True
            nc.sync.dma_start(w1g[:, 0, 0], w1p[0, :, 0])
            nc.sync.dma_start(w1g[:, 0, 1:NFB], w1p[0, :, 1:NFB])
            if len(sched) > 1:
                xg_q.append(load_xg(1))
            load_w2(0)
            if len(sched) > 2:
                xg_q.append(load_xg(2))
            PREF = 3  # xg prefetch depth (xpool bufs = PREF + 1)

            for si, (e, row0, xoff, sz, nx) in enumerate(sched):
                if si == 0 or sched[si - 1][0] != e:
                    load_w1(e)  # safety: must be resident now
                    load_w2(e)
                    seg_chunk = 0
                else:
                    seg_chunk += 1
                nt = sz // P  # mm2/retire always use full 128-token tiles:
                # partial-height DMAs degrade to one engine — pad hT instead
                xg_s = xg_q.pop(0)
                if si + PREF < len(sched):
                    xg_q.append(load_xg(si + PREF))
                if e + 1 < E:
                    if seg_chunk == 0:
                        load_w1(e + 1)
                    elif seg_chunk == 1:
                        load_w2(e + 1)

                yps = [
                    psum_y.tile([P, C], f32, tag=f"y_{t}", name=f"y_{t}")
                    for t in range(nt)
                ]

                final_chunk = si == len(sched) - 1

                def retire_tile(t, row0=row0, yps=yps, split=False):
                    r = row0 + t * P
                    yf = ypool.tile([P, C], bf16, tag=f"yf{t % 2}", name="yf")
                    if split:
                        # tail shave for the very last tile: copy each
                        # 512-col half on a separate engine (DVE || ACT) as
                        # soon as ITS accumulation closes, one full DMA
                        nc.vector.tensor_copy(yf[:, 0:512], yps[t][:, 0:512])
                        nc.scalar.activation(
                            yf[:, 512:C], yps[t][:, 512:C],
                            mybir.ActivationFunctionType.Copy,
                        )
                    elif t % 2 == 0:
                        # single copy per tile, alternating DVE / ACT so
                        # neighboring tiles drain in parallel
                        nc.vector.tensor_copy(yf[:], yps[t][:])
                    else:
                        nc.scalar.activation(
                            yf[:], yps[t][:], mybir.ActivationFunctionType.Copy
                        )
                    nc.sync.dma_start(yg[r : r + P, :], yf[:])

                def mm2(fl, hT, last=False, yps=yps, nt=nt, e=e,
                        final_chunk=final_chunk):
                    for t in range(nt):
                        split = last and final_chunk and t == nt - 1
                        for cc in range(2):
                            nc.tensor.matmul(
                                yps[t][:, cc * 512 : (cc + 1) * 512],
                                lhsT=hT[:, t * P : (t + 1) * P],
                                rhs=w2g[:, e, fl, cc * 512 : (cc + 1) * 512],
                                start=(fl == 0),
                                stop=(fl == NFB - 1),
                            )
                        if last:
                            # retire as soon as this tile's accumulation
                            # closes: frees its PSUM banks for the next
                            # chunk's mm2 that much earlier
                            retire_tile(t, split=split)

                # software pipeline: mm2 runs two fl behind mm1, so the
                # relu feeding each mm2 block retired long before the PE
                # reaches it; the last two mm2 blocks + the PSUM retires
                # drain after the next chunk's first mm1 blocks.  Chunk 0
                # lags a full NFB so its mm2 (and so its w2 slices) start
                # only after all of mm1 — w2 bytes stay out of the scarce
                # cold-DMA window.
                lag = NFB if si == 0 else 2
                hTs = []
                for fl in range(NFB):
                    hps = psum_h.tile([P, SCH], f32, tag="h", name="hps")
                    for c in range(8):
                        nc.tensor.matmul(
                            hps[:, :nx],
                            lhsT=w1g[:, e, fl, c, :],
                            rhs=xg_s[:, c, :],
                            start=(c == 0),
                            stop=(c == 7),
                        )
                    hT = hpool.tile([P, SCH], bf16, tag="hT", name="hT")
                    if nx < sz:
                        # runt chunk: mm1 streamed only the nx real tokens;
                        # zero hT's pad columns so mm2 runs full 128-tiles
                        # (partial-height DMAs/psum would be slower)
                        nc.vector.memset(hT[:, nx:sz], 0.0)
                    if fl >= NFB - 2:
                        # last fl's: per-token-tile relu so mm2(t) can
                        # start as soon as its slice is ready
                        for t in range(nt):
                            tl = slice(t * P, min((t + 1) * P, nx))
                            if tl.start >= nx:
                                break
                            nc.scalar.activation(
                                hT[:, tl],
                                hps[:, tl],
                                mybir.ActivationFunctionType.Relu,
                            )
                    else:
                        nc.scalar.activation(
                            hT[:, :nx],
                            hps[:, :nx],
                            mybir.ActivationFunctionType.Relu,
                        )
                    hTs.append(hT)
                    if fl >= lag:
                        mm2(fl - lag, hTs[fl - lag])
                for k in range(NFB - min(lag, NFB), NFB):
                    mm2(k, hTs[k], last=(k == NFB - 1))
    nc.compile()
    return nc


_CACHE = {}
_TRACE = False  # test harness sets True to capture an NTFF profile
_LAST_RES = None


def _get_nc(key, builder):
    if key not in _CACHE:
        _CACHE[key] = builder()
    return _CACHE[key]


def _route(x_flat, router_w):
    """Top-2 routing, float64 for stable selection. Returns idx/weights per expert."""
    logits = x_flat.astype(np.float64) @ router_w.astype(np.float64).T
    t = np.exp(logits - logits.max(-1, keepdims=True))
    p = t / t.sum(-1, keepdims=True)
    top2 = np.argsort(-p, axis=-1)[:, :2]
    pv = np.take_along_axis(p, top2, axis=-1)
    wn = pv / (pv.sum(-1, keepdims=True) + 1e-9)
    return top2, wn


def kernel(x, router_w, w1, b1, w2, b2):
    import ml_dtypes
    from concourse.bass_utils import run_bass_kernel_spmd

    bf16 = ml_dtypes.bfloat16
    Bx, Nx, Cx = x.shape
    x_flat = np.ascontiguousarray(x.reshape(-1, Cx))
    T = x_flat.shape[0]

    top2, wn = _route(x_flat, router_w)
    idxs, gates = [], []
    for e in range(E):
        sel = top2 == e
        we = np.where(sel, wn, 0.0).sum(-1)
        idx = np.nonzero(sel.any(-1))[0]
        idxs.append(idx)
        gates.append(we[idx].astype(np.float32))

    fast = bool(np.all(b1 == 0) and np.all(b2 == 0))
    global _LAST_RES

    if not fast:
        cap = max(len(i) for i in idxs)
        cap = ((cap + SCH - 1) // SCH) * SCH
        nc = _get_nc(("slow", cap), lambda: _build(cap))
        in_maps = []
        for e in range(E):
            n_e = len(idxs[e])
            xg = np.zeros((cap, Cx), np.float32)
            xg[:n_e] = x_flat[idxs[e]]
            wg = np.zeros(cap, np.float32)
            wg[:n_e] = gates[e]
            in_maps.append(
                {
                    "xgT": np.ascontiguousarray(xg.T),
                    "w1t": np.ascontiguousarray(w1[e].T),
                    "w2t": np.ascontiguousarray(w2[e].T),
                    "b1r": np.ascontiguousarray(b1[e].reshape(F // P, P).T),
                    "b2r": np.ascontiguousarray(np.broadcast_to(b2[e], (P, Cx))),
                    "wg": np.ascontiguousarray(wg.reshape(cap // P, P).T),
                }
            )
        res = run_bass_kernel_spmd(nc, in_maps, core_ids=list(range(E)), trace=_TRACE)
        _LAST_RES = res
        out = np.zeros((T, Cx), np.float32)
        for e in range(E):
            n_e = len(idxs[e])
            out[idxs[e]] += res.results[e]["yg"][:n_e].astype(np.float32)
        return out.reshape(Bx, Nx, Cx)

    # ---- fast path: F-parallel over all experts ----
    # order experts so the very last chunk is as small as possible (the
    # final retire + output DMA is the kernel's tail)
    ms_nat = [((len(idxs[e]) + P - 1) // P) * P for e in range(E)]
    order = sorted(range(E), key=lambda e: (-(ms_nat[e] % SCH or SCH), e))
    ms = tuple(ms_nat[e] for e in order)
    ns = tuple(len(idxs[e]) for e in order)
    nc = _get_nc(("fast", ms, ns), lambda: _build_fast(ms, ns))

    # shared gated token stream, expert-major, per-chunk [ci, co, n] tiles,
    # chunk blocks packed tight (exact sizes, no pad rows)
    blocks = []
    for i, e in enumerate(order):
        n_e = len(idxs[e])
        xgb = (x_flat[idxs[e]] * gates[e][:, None]).astype(bf16)  # pre-gate
        off = 0
        left = n_e
        for sz in _chunks(ms_nat[e]):
            nx = min(left, sz)
            blocks.append(
                np.ascontiguousarray(
                    xgb[off : off + nx].reshape(nx, 8, P).transpose(2, 1, 0)
                ).ravel()
            )
            off += nx
            left -= nx
    xgf = np.concatenate(blocks)

    # per-core weight slices: core d holds f-blocks [4d, 4d+4) of every expert
    # w1 tiled:  w1t[e][fb, fo, c, ci] -> lhsT layout [ci, c, fo]
    w1t = [
        w1[e].reshape(F // P, P, 8, P).transpose(0, 3, 2, 1).astype(bf16)
        for e in order
    ]
    w2t = [w2[e].T.reshape(F // P, P, Cx).astype(bf16) for e in order]
    in_maps = []
    for d in range(8):
        fsl = slice(NFB * d, NFB * (d + 1))
        w1pd = np.ascontiguousarray(
            np.stack([w1t[i][fsl].transpose(1, 0, 2, 3) for i in range(E)])
        )
        w2pd = np.ascontiguousarray(
            np.stack([w2t[i][fsl].transpose(1, 0, 2) for i in range(E)])
        )
        in_maps.append({"xgf": xgf, "w1p": w1pd, "w2p": w2pd})

    res = run_bass_kernel_spmd(nc, in_maps, core_ids=list(range(8)), trace=_TRACE)
    _LAST_RES = res

    # host combine: sum the 8 partial-y streams, then scatter-add per expert
    ysum = res.results[0]["yg"].astype(np.float32)
    for d in range(1, 8):
        ysum += res.results[d]["yg"].astype(np.float32)
    out = np.zeros((T, Cx), np.float32)
    off = 0
    for i, e in enumerate(order):
        n_e = len(idxs[e])
        out[idxs[e]] += ysum[off : off + n_e]
        off += ms[i]
    return out.reshape(Bx, Nx, Cx)
